# revision 36
# baseline (speedup 1.0000x reference)
"""Trainium2 Bass kernel for the MACE-style symmetric contraction:

    out  = einsum("xyik,kc,bci->bcxy", U3, w3, nf)
    c2   = einsum("xyk,kc->cxy", U2, w2)[None] + out
    out  = einsum("bcxi,bci->bcx", c2, nf)
    c1   = einsum("xk,kc->cx", U1, w1)[None] + out
    out  = einsum("bci,bci->bc", c1, nf)

Algebraically this is

    out[b,c] =   sum_{x,y,i} W3U[x,y,i,c] nf[b,c,x] nf[b,c,y] nf[b,c,i]
               + sum_{x,y}   U2w2[c,x,y]  nf[b,c,x] nf[b,c,y]
               + sum_{x}     U1w1[c,x]    nf[b,c,x]

with W3U = einsum("xyik,kc->xyic", U3, w3).  We fold the U2/U1 terms into
the triple product by augmenting the i axis (row i'=48 holds U2w2, entry
(i'=48, y'=48) holds U1w1) and appending a constant-1 channel to nf.

Sharding: the leading irrep axis x (48) is split 6-per-core across the 8
NeuronCores (this also splits the dominant HBM traffic, U3 = 562 MB, and
the W3U build FLOPs 8 ways).  Each core computes a partial [512, 96]
output (its x-slice of the outer sums); the host adds the 8 partials.

Per-core device pipeline:
  1. build W3Ua[c, m]  (m = (i,x,y') = 48*6*49) = u3t.T @ w3 on TensorE,
     PSUM -> DRAM scratch.
  2. re-layout per c-pair via DRAM->SBUF DMA into [i'=49(x2), (x,y')=294]
     tiles (partition rows 0:49 / 64:113).
  3. per (c, b-chunk of 128): one matmul  Z[b,(x,y')] = nfa_c.T @ W3Ua_c
     (contract i'), one VectorE multiply forming P2[b,(x,y')] =
     nf_x * nfa_y', and one fused tensor_tensor_reduce
     sum_{x,y'}(Z * P2) -> out[b, c].
"""

import numpy as np

B = 512          # atoms
C = 96           # feats
I = 48           # irreps
K3, K2, K1 = 1270, 24, 3
NCORES = 8
XS = I // NCORES  # 6 x-values per core
Y1 = I + 1        # 49: y plus augmentation column
I1 = I + 1        # 49: i plus augmentation row
KP = 1280         # K3 padded to 10 partition tiles
NX = XS * Y1      # 294
MP = I * XS * Y1  # 14112  (m = (i, x, y'), i outermost)
MCHUNK = 3 * NX   # 882: whole i-rows per chunk (16 chunks exactly)
NMC = MP // MCHUNK                 # 16
KT = KP // 128                     # 10
PAIRS = C // 2                     # 48
NF = 55                            # per-c nf pack: 49 (nfa_y) + 6 (nf_x)

_CACHE = {}

# exec time of the last device run (ns), when BASS_TRACE=1
LAST_EXEC_NS = None


def _build_nc(debug=None):
    import concourse.bass as bass
    import concourse.mybir as mybir
    from concourse.tile import TileContext

    f32 = mybir.dt.float32
    bf16 = mybir.dt.bfloat16
    mult = mybir.AluOpType.mult
    add = mybir.AluOpType.add

    import concourse.bacc as bacc
    nc = bacc.Bacc(None, target_bir_lowering=False)
    u3t = nc.dram_tensor("u3t", [KP, MP], bf16, kind="ExternalInput")
    w3p = nc.dram_tensor("w3p", [KP, C], bf16, kind="ExternalInput")
    nfa = nc.dram_tensor("nfa", [128, PAIRS * B], bf16, kind="ExternalInput")
    nfy = nc.dram_tensor("nfy", [B, C * I1], bf16, kind="ExternalInput")
    nfx2 = nc.dram_tensor("nfx2", [B, C * XS], f32, kind="ExternalInput")
    u2aug = nc.dram_tensor("u2aug", [32, NX], f32, kind="ExternalInput")
    w21 = nc.dram_tensor("w21", [32, C], f32, kind="ExternalInput")
    outp = nc.dram_tensor("out", [B, C], f32, kind="ExternalOutput")

    with TileContext(nc) as tc:
        with (
            tc.tile_pool(name="dram", bufs=1, space="DRAM") as dpool,
            tc.tile_pool(name="const", bufs=1) as cpool,
            tc.tile_pool(name="u3", bufs=14) as u3pool,
            tc.tile_pool(name="bpsum", bufs=4, space="PSUM") as bpsum,
            tc.tile_pool(name="zpsum", bufs=2, space="PSUM") as zpsum,
            tc.tile_pool(name="lt", bufs=4) as ltpool,
            tc.tile_pool(name="zsb", bufs=4) as zsbpool,
            tc.tile_pool(name="p2", bufs=4) as p2pool,
            tc.tile_pool(name="stg", bufs=3) as stgpool,
        ):
            # scratch laid out for phase-B consumption: per c-pair a
            # 128-row block (row = 64*(c%2) + i'), xy contiguous.
            w3u_scr = dpool.tile([PAIRS * 128, NX], bf16)
            scr_v = w3u_scr[:, :].rearrange("(cp c2 q) x -> cp c2 q x",
                                            c2=2, q=64)

            # ---- resident constants ----
            w3sb = cpool.tile([128, KT * C], bf16)
            w3v = w3sb[:, :].rearrange("p (k c) -> p k c", c=C)
            nc.sync.dma_start(
                out=w3v[:, :, :],
                in_=w3p[:, :].rearrange("(k p) c -> p k c", p=128))
            nfasb = cpool.tile([128, PAIRS * B], bf16)
            nc.sync.dma_start(out=nfasb[:, :], in_=nfa[:, :])
            nfav = nfasb[:, :].rearrange("p (cp b) -> p cp b", b=B)
            w21sb = cpool.tile([32, C], f32)
            nc.sync.dma_start(out=w21sb[:, :], in_=w21[:, :])
            u2sb = cpool.tile([32, NX], f32)
            nc.sync.dma_start(out=u2sb[:, :], in_=u2aug[:, :])
            NT = B // 128
            nfyts = [cpool.tile([128, C * I1], bf16, tag=f"nfy{t}",
                                name=f"nfy{t}") for t in range(NT)]
            for t in range(NT):
                nc.sync.dma_start(out=nfyts[t][:, :],
                                  in_=nfy[t * 128:(t + 1) * 128, :])
            ybufs = [cpool.tile([128, C * XS], f32, tag=f"yb{t}",
                                name=f"yb{t}") for t in range(NT)]
            nfx2ts = [cpool.tile([128, C * XS], f32, tag=f"nfx2{t}",
                                 name=f"nfx2{t}") for t in range(NT)]
            for t in range(NT):
                nc.sync.dma_start(out=nfx2ts[t][:, :],
                                  in_=nfx2[t * 128:(t + 1) * 128, :])

            # ---- aug build: [96, 294] = w21.T @ u2aug -> i'=48 rows ----
            aps = bpsum.tile([C, MCHUNK // 2], f32, tag="bp")
            nc.tensor.matmul(aps[:, :NX], w21sb[:27, :], u2sb[:27, :],
                             start=True, stop=True)
            astg = stgpool.tile([C, MCHUNK], bf16, tag="stg")
            nc.scalar.copy(astg[:, :NX], aps[:, :NX])
            # stg partitions are c2-major (w3p cols permuted host-side), so
            # each c2 half drains with a plain single-partition-dim DMA
            for c2 in range(2):
                nc.sync.dma_start(
                    out=scr_v[:, c2, I, :],
                    in_=astg[48 * c2:48 * (c2 + 1), :NX])

            # ---- W3U build: [96, 14112] = w3p.T @ u3t, k-accumulated ----
            # each chunk covers 3 whole i'-rows (882 = 3*294) so the drain
            # scatters straight into the per-c-pair scratch blocks
            MH = MCHUNK // 2  # 441: matmul free dim must stay <= 512
            for mc in range(NMC):
                subs = []
                for j in range(KT // 2):
                    tj = u3pool.tile([128, 2 * MCHUNK], bf16, tag="u3")
                    nc.sync.dma_start(
                        out=tj[:, :].rearrange("p (k m) -> p k m", k=2),
                        in_=u3t[256 * j:256 * (j + 1),
                                mc * MCHUNK:(mc + 1) * MCHUNK].rearrange(
                                    "(k p) m -> p k m", p=128))
                    subs.append(tj)
                stg = stgpool.tile([C, MCHUNK], bf16, tag="stg")
                for h in range(2):
                    ps = bpsum.tile([C, MH], f32, tag="bp",
                                    name=f"bp{mc}h{h}")
                    for kt in range(KT):
                        tv_ = subs[kt // 2][:, :].rearrange(
                            "p (k m) -> p k m", k=2)[
                            :, kt % 2, h * MH:(h + 1) * MH]
                        nc.tensor.matmul(ps[:, :], w3v[:, kt, :], tv_,
                                         start=(kt == 0),
                                         stop=(kt == KT - 1))
                    if mc % 2 == 0:
                        nc.scalar.copy(stg[:, h * MH:(h + 1) * MH],
                                       ps[:, :])
                    else:
                        nc.vector.tensor_copy(
                            stg[:, h * MH:(h + 1) * MH], ps[:, :])
                for c2 in range(2):
                    nc.sync.dma_start(
                        out=scr_v[:, c2, 3 * mc:3 * mc + 3, :],
                        in_=stg[48 * c2:48 * (c2 + 1), :].rearrange(
                            "cp (i x) -> cp i x", x=NX))

            npairs = 0 if debug == "A" else (
                debug if isinstance(debug, int) else PAIRS)
            # engine split: Scalar drains Z to bf16 SBUF (GpSimd cannot
            # read PSUM), the multiply runs on GpSimd for 3 of 4 c-pairs
            # and on DVE (2x bf16 mode) for the rest, DVE does every
            # reduce.
            for cpp in range((npairs + 1) // 2):
                lt = ltpool.tile([128, 2 * NX], bf16, tag="lt")
                ltv = lt[:, :].rearrange("p (cpx x) -> p cpx x", x=NX)
                nc.sync.dma_start(
                    out=ltv,
                    in_=w3u_scr[256 * cpp:256 * (cpp + 1), :].rearrange(
                        "(cpx r) x -> r cpx x", cpx=2))
                for cpx in range(2):
                    cp = 2 * cpp + cpx
                    if cp >= npairs:
                        break
                    c0 = 2 * cp
                    for t in range(NT):
                        nfyv = nfyts[t][:, :].rearrange(
                            "p (c i) -> p c i", i=I1)[:, c0:c0 + 2, :]
                        zt = zpsum.tile([128, 1024], f32, tag="z")
                        for ci in range(2):
                            lhsT = nfav[64 * ci:64 * ci + I1, cp,
                                        t * 128:(t + 1) * 128]
                            nc.tensor.matmul(
                                zt[:, 512 * ci:512 * ci + NX], lhsT,
                                ltv[64 * ci:64 * ci + I1, cpx, :],
                                start=True, stop=True)
                        zsb = zsbpool.tile([128, 2 * NX], bf16, tag="zsb")
                        nc.scalar.copy(zsb[:, :], zt[:, :].rearrange(
                            "p (c n) -> p c n", n=512)[:, :, 0:NX])
                        zsv = zsb[:, :].rearrange("p (c x y) -> p c x y",
                                                  c=2, y=Y1)
                        tmp = p2pool.tile([128, 2 * NX], bf16, tag="p2")
                        tv = tmp[:, :].rearrange("p (c x y) -> p c x y",
                                                 c=2, y=Y1)
                        tt_eng = nc.vector if cp % 4 == 3 else nc.gpsimd
                        tt_eng.tensor_tensor(
                            tv, zsv,
                            nfyv[:, :, None, :].to_broadcast(
                                [128, 2, XS, Y1]),
                            mult)
                        nc.vector.tensor_reduce(
                            ybufs[t][:, cp * 2 * XS:(cp + 1) * 2 * XS],
                            tv, axis=mybir.AxisListType.X, op=add)
            if debug != "A":
                for t in range(NT):
                    yn = p2pool.tile([128, C * XS], f32, tag="yn")
                    nc.vector.tensor_tensor(yn[:, :], ybufs[t][:, :],
                                            nfx2ts[t][:, :], mult)
                    ostf = p2pool.tile([128, C], f32, tag="ostf")
                    nc.vector.tensor_reduce(
                        ostf[:, :],
                        yn[:, :].rearrange("p (c x) -> p c x", x=XS),
                        axis=mybir.AxisListType.X, op=add)
                    nc.sync.dma_start(out=outp[t * 128:(t + 1) * 128, :],
                                      in_=ostf[:, :])
    nc.finalize()
    return nc


def _prep_inputs(node_feats, w3, w2, w1, U3, U2, U1):
    """Host-side sharding / re-layout. No reference contractions are done
    here -- only transposes, padding, concatenation and dtype casts of the
    raw inputs."""
    import ml_dtypes
    bf16 = ml_dtypes.bfloat16
    f32 = np.float32
    node_feats = np.ascontiguousarray(np.asarray(node_feats, dtype=f32))
    w3 = np.asarray(w3, dtype=f32)
    w2 = np.asarray(w2, dtype=f32)
    w1 = np.asarray(w1, dtype=f32)
    U3 = np.asarray(U3, dtype=f32)
    U2 = np.asarray(U2, dtype=f32)
    U1 = np.asarray(U1, dtype=f32)

    # shared across cores.  w3p/w21 columns are permuted to c2-major order
    # (c' = (c%2)*48 + c//2) so the phase-A PSUM partitions line up with
    # the per-c-pair scratch blocks without splitting the partition dim.
    cperm = np.array([2 * (cc % PAIRS) + cc // PAIRS for cc in range(C)])
    w3p = np.zeros((KP, C), dtype=bf16)
    w3p[:K3] = w3
    w3p = np.ascontiguousarray(w3p[:, cperm])
    w21 = np.zeros((32, C), dtype=f32)
    w21[:K2] = w2
    w21[K2:K2 + K1] = w1
    w21 = np.ascontiguousarray(w21[:, cperm])

    # nfa: [p, cp, b]; p = 64*(c%2) + i'; i'=48 row is the ones channel
    nfT = node_feats.transpose(1, 2, 0)  # [c, i, b]
    nfa = np.zeros((128, PAIRS, B), dtype=f32)
    for par in (0, 1):
        nfa[64 * par:64 * par + I] = nfT[par::2].transpose(1, 0, 2)
        nfa[64 * par + I] = 1.0
    nfa = np.ascontiguousarray(nfa.reshape(128, PAIRS * B)).astype(bf16)

    U3h = U3.astype(bf16)  # cast once; per-core slices below stay bf16
    in_maps = []
    for r in range(NCORES):
        xlo = XS * r
        # u3t: [k, m], m = (i, x, y') with zero-padded y'=48 col and k pad
        u3s = U3h[xlo:xlo + XS]                      # [6, 48, 48, 1270]
        u3a = np.zeros((I, XS, Y1, KP), dtype=bf16)  # [i, x, y', k]
        u3a[:, :, :I, :K3] = u3s.transpose(2, 0, 1, 3)
        u3t = np.ascontiguousarray(u3a.reshape(MP, KP).T)

        # u2aug: rows 0:24 U2 slice, rows 24:27 U1 slice (at y'=48)
        u2a = np.zeros((32, XS, Y1), dtype=f32)
        u2a[:K2, :, :I] = U2[xlo:xlo + XS].transpose(2, 0, 1)
        u2a[K2:K2 + K1, :, I] = U1[xlo:xlo + XS].T
        u2a = np.ascontiguousarray(u2a.reshape(32, NX))

        # nfy: [b, c, 49] = nf with ones channel; nfx2: [b, c, 6] x-slice
        nfy = np.empty((B, C, I1), dtype=bf16)
        nfy[:, :, :I] = node_feats
        nfy[:, :, I] = 1.0
        nfy = np.ascontiguousarray(nfy.reshape(B, C * I1))
        nfx2 = np.ascontiguousarray(
            node_feats[:, :, xlo:xlo + XS].reshape(B, C * XS))

        in_maps.append({
            "u3t": u3t,
            "w3p": w3p,
            "nfa": nfa,
            "nfy": nfy,
            "nfx2": nfx2,
            "u2aug": u2a,
            "w21": w21,
        })
    return in_maps


def kernel(node_feats, w3, w2, w1, U3, U2, U1):
    global LAST_EXEC_NS
    import os
    from concourse.bass_utils import run_bass_kernel_spmd

    if "nc" not in _CACHE:
        _CACHE["nc"] = _build_nc()
    nc = _CACHE["nc"]

    in_maps = _prep_inputs(node_feats, w3, w2, w1, U3, U2, U1)
    trace = bool(os.environ.get("BASS_TRACE"))
    res = run_bass_kernel_spmd(nc, in_maps, list(range(NCORES)), trace=trace)
    LAST_EXEC_NS = res.exec_time_ns
    _CACHE["last_results"] = res

    out = np.zeros((B, C), dtype=np.float64)
    for r in range(NCORES):
        out += res.results[r]["out"].astype(np.float64)
    return out.astype(np.float32)



# revision 37
# speedup vs baseline: 1.0056x; 1.0056x over previous
"""Trainium2 Bass kernel for the MACE-style symmetric contraction:

    out  = einsum("xyik,kc,bci->bcxy", U3, w3, nf)
    c2   = einsum("xyk,kc->cxy", U2, w2)[None] + out
    out  = einsum("bcxi,bci->bcx", c2, nf)
    c1   = einsum("xk,kc->cx", U1, w1)[None] + out
    out  = einsum("bci,bci->bc", c1, nf)

Algebraically this is

    out[b,c] =   sum_{x,y,i} W3U[x,y,i,c] nf[b,c,x] nf[b,c,y] nf[b,c,i]
               + sum_{x,y}   U2w2[c,x,y]  nf[b,c,x] nf[b,c,y]
               + sum_{x}     U1w1[c,x]    nf[b,c,x]

with W3U = einsum("xyik,kc->xyic", U3, w3).  We fold the U2/U1 terms into
the triple product by augmenting the i axis (row i'=48 holds U2w2, entry
(i'=48, y'=48) holds U1w1) and appending a constant-1 channel to nf.

Sharding: the leading irrep axis x (48) is split 6-per-core across the 8
NeuronCores (this also splits the dominant HBM traffic, U3 = 562 MB, and
the W3U build FLOPs 8 ways).  Each core computes a partial [512, 96]
output (its x-slice of the outer sums); the host adds the 8 partials.

Per-core device pipeline:
  1. build W3Ua[c, m]  (m = (i,x,y') = 48*6*49) = u3t.T @ w3 on TensorE,
     PSUM -> DRAM scratch.
  2. re-layout per c-pair via DRAM->SBUF DMA into [i'=49(x2), (x,y')=294]
     tiles (partition rows 0:49 / 64:113).
  3. per (c, b-chunk of 128): one matmul  Z[b,(x,y')] = nfa_c.T @ W3Ua_c
     (contract i'), one VectorE multiply forming P2[b,(x,y')] =
     nf_x * nfa_y', and one fused tensor_tensor_reduce
     sum_{x,y'}(Z * P2) -> out[b, c].
"""

import numpy as np

B = 512          # atoms
C = 96           # feats
I = 48           # irreps
K3, K2, K1 = 1270, 24, 3
NCORES = 8
XS = I // NCORES  # 6 x-values per core
Y1 = I + 1        # 49: y plus augmentation column
I1 = I + 1        # 49: i plus augmentation row
KP = 1280         # K3 padded to 10 partition tiles
NX = XS * Y1      # 294
MP = I * XS * Y1  # 14112  (m = (i, x, y'), i outermost)
MCHUNK = 3 * NX   # 882: whole i-rows per chunk (16 chunks exactly)
NMC = MP // MCHUNK                 # 16
KT = KP // 128                     # 10
PAIRS = C // 2                     # 48
NF = 55                            # per-c nf pack: 49 (nfa_y) + 6 (nf_x)

_CACHE = {}

# exec time of the last device run (ns), when BASS_TRACE=1
LAST_EXEC_NS = None


def _build_nc(debug=None):
    import concourse.bass as bass
    import concourse.mybir as mybir
    from concourse.tile import TileContext

    f32 = mybir.dt.float32
    bf16 = mybir.dt.bfloat16
    mult = mybir.AluOpType.mult
    add = mybir.AluOpType.add

    import concourse.bacc as bacc
    nc = bacc.Bacc(None, target_bir_lowering=False)
    u3t = nc.dram_tensor("u3t", [KP, MP], bf16, kind="ExternalInput")
    w3p = nc.dram_tensor("w3p", [KP, C], bf16, kind="ExternalInput")
    nfa = nc.dram_tensor("nfa", [128, PAIRS * B], bf16, kind="ExternalInput")
    nfy = nc.dram_tensor("nfy", [B, C * I1], bf16, kind="ExternalInput")
    nfx2 = nc.dram_tensor("nfx2", [B, C * XS], f32, kind="ExternalInput")
    u2aug = nc.dram_tensor("u2aug", [32, NX], f32, kind="ExternalInput")
    w21 = nc.dram_tensor("w21", [32, C], f32, kind="ExternalInput")
    outp = nc.dram_tensor("out", [B, C], f32, kind="ExternalOutput")

    with TileContext(nc) as tc:
        with (
            tc.tile_pool(name="dram", bufs=1, space="DRAM") as dpool,
            tc.tile_pool(name="const", bufs=1) as cpool,
            tc.tile_pool(name="u3", bufs=10) as u3pool,
            tc.tile_pool(name="bpsum", bufs=4, space="PSUM") as bpsum,
            tc.tile_pool(name="zpsum", bufs=2, space="PSUM") as zpsum,
            tc.tile_pool(name="lt", bufs=3) as ltpool,
            tc.tile_pool(name="zsb", bufs=3) as zsbpool,
            tc.tile_pool(name="p2", bufs=3) as p2pool,
            tc.tile_pool(name="stg", bufs=3) as stgpool,
        ):
            # scratch laid out for phase-B consumption: per c-pair a
            # 128-row block (row = 64*(c%2) + i'), xy contiguous.
            w3u_scr = dpool.tile([PAIRS * 128, NX], bf16)
            scr_v = w3u_scr[:, :].rearrange("(cp c2 q) x -> cp c2 q x",
                                            c2=2, q=64)

            # ---- resident constants ----
            w3sb = cpool.tile([128, KT * C], bf16)
            w3v = w3sb[:, :].rearrange("p (k c) -> p k c", c=C)
            nc.sync.dma_start(
                out=w3v[:, :, :],
                in_=w3p[:, :].rearrange("(k p) c -> p k c", p=128))
            nfasb = cpool.tile([128, PAIRS * B], bf16)
            nc.sync.dma_start(out=nfasb[:, :], in_=nfa[:, :])
            nfav = nfasb[:, :].rearrange("p (cp b) -> p cp b", b=B)
            w21sb = cpool.tile([32, C], f32)
            nc.sync.dma_start(out=w21sb[:, :], in_=w21[:, :])
            u2sb = cpool.tile([32, NX], f32)
            nc.sync.dma_start(out=u2sb[:, :], in_=u2aug[:, :])
            NT = B // 128
            nfyts = [cpool.tile([128, C * I1], bf16, tag=f"nfy{t}",
                                name=f"nfy{t}") for t in range(NT)]
            for t in range(NT):
                nc.sync.dma_start(out=nfyts[t][:, :],
                                  in_=nfy[t * 128:(t + 1) * 128, :])
            ybufs = [cpool.tile([128, C * XS], f32, tag=f"yb{t}",
                                name=f"yb{t}") for t in range(NT)]
            nfx2ts = [cpool.tile([128, C * XS], f32, tag=f"nfx2{t}",
                                 name=f"nfx2{t}") for t in range(NT)]
            for t in range(NT):
                nc.sync.dma_start(out=nfx2ts[t][:, :],
                                  in_=nfx2[t * 128:(t + 1) * 128, :])

            # ---- aug build: [96, 294] = w21.T @ u2aug -> i'=48 rows ----
            aps = bpsum.tile([C, MCHUNK // 2], f32, tag="bp")
            nc.tensor.matmul(aps[:, :NX], w21sb[:27, :], u2sb[:27, :],
                             start=True, stop=True)
            astg = stgpool.tile([C, MCHUNK], bf16, tag="stg")
            nc.scalar.copy(astg[:, :NX], aps[:, :NX])
            # stg partitions are c2-major (w3p cols permuted host-side), so
            # each c2 half drains with a plain single-partition-dim DMA
            for c2 in range(2):
                nc.sync.dma_start(
                    out=scr_v[:, c2, I, :],
                    in_=astg[48 * c2:48 * (c2 + 1), :NX])

            # ---- W3U build: [96, 14112] = w3p.T @ u3t, k-accumulated ----
            # each chunk covers 3 whole i'-rows (882 = 3*294) so the drain
            # scatters straight into the per-c-pair scratch blocks
            MH = MCHUNK // 2  # 441: matmul free dim must stay <= 512
            for mc in range(NMC):
                subs = []
                for j in range(KT // 2):
                    tj = u3pool.tile([128, 2 * MCHUNK], bf16, tag="u3")
                    nc.sync.dma_start(
                        out=tj[:, :].rearrange("p (k m) -> p k m", k=2),
                        in_=u3t[256 * j:256 * (j + 1),
                                mc * MCHUNK:(mc + 1) * MCHUNK].rearrange(
                                    "(k p) m -> p k m", p=128))
                    subs.append(tj)
                stg = stgpool.tile([C, MCHUNK], bf16, tag="stg")
                for h in range(2):
                    ps = bpsum.tile([C, MH], f32, tag="bp",
                                    name=f"bp{mc}h{h}")
                    for kt in range(KT):
                        tv_ = subs[kt // 2][:, :].rearrange(
                            "p (k m) -> p k m", k=2)[
                            :, kt % 2, h * MH:(h + 1) * MH]
                        nc.tensor.matmul(ps[:, :], w3v[:, kt, :], tv_,
                                         start=(kt == 0),
                                         stop=(kt == KT - 1))
                    if mc % 2 == 0:
                        nc.scalar.copy(stg[:, h * MH:(h + 1) * MH],
                                       ps[:, :])
                    else:
                        nc.vector.tensor_copy(
                            stg[:, h * MH:(h + 1) * MH], ps[:, :])
                for c2 in range(2):
                    nc.sync.dma_start(
                        out=scr_v[:, c2, 3 * mc:3 * mc + 3, :],
                        in_=stg[48 * c2:48 * (c2 + 1), :].rearrange(
                            "cp (i x) -> cp i x", x=NX))

            npairs = 0 if debug == "A" else (
                debug if isinstance(debug, int) else PAIRS)
            # engine split: Scalar drains Z to bf16 SBUF (GpSimd cannot
            # read PSUM), the multiply runs on GpSimd for 3 of 4 c-pairs
            # and on DVE (2x bf16 mode) for the rest, DVE does every
            # reduce.
            for cpp in range((npairs + 1) // 2):
                lt = ltpool.tile([128, 2 * NX], bf16, tag="lt")
                ltv = lt[:, :].rearrange("p (cpx x) -> p cpx x", x=NX)
                nc.sync.dma_start(
                    out=ltv,
                    in_=w3u_scr[256 * cpp:256 * (cpp + 1), :].rearrange(
                        "(cpx r) x -> r cpx x", cpx=2))
                for cpx in range(2):
                    cp = 2 * cpp + cpx
                    if cp >= npairs:
                        break
                    c0 = 2 * cp
                    for t in range(NT):
                        nfyv = nfyts[t][:, :].rearrange(
                            "p (c i) -> p c i", i=I1)[:, c0:c0 + 2, :]
                        zt = zpsum.tile([128, 1024], f32, tag="z")
                        for ci in range(2):
                            lhsT = nfav[64 * ci:64 * ci + I1, cp,
                                        t * 128:(t + 1) * 128]
                            nc.tensor.matmul(
                                zt[:, 512 * ci:512 * ci + NX], lhsT,
                                ltv[64 * ci:64 * ci + I1, cpx, :],
                                start=True, stop=True)
                        zsb = zsbpool.tile([128, 2 * NX], bf16, tag="zsb")
                        nc.scalar.copy(zsb[:, :], zt[:, :].rearrange(
                            "p (c n) -> p c n", n=512)[:, :, 0:NX])
                        zsv = zsb[:, :].rearrange("p (c x y) -> p c x y",
                                                  c=2, y=Y1)
                        tmp = p2pool.tile([128, 2 * NX], bf16, tag="p2")
                        tv = tmp[:, :].rearrange("p (c x y) -> p c x y",
                                                 c=2, y=Y1)
                        tt_eng = nc.vector if cp % 4 == 3 else nc.gpsimd
                        tt_eng.tensor_tensor(
                            tv, zsv,
                            nfyv[:, :, None, :].to_broadcast(
                                [128, 2, XS, Y1]),
                            mult)
                        nc.vector.tensor_reduce(
                            ybufs[t][:, cp * 2 * XS:(cp + 1) * 2 * XS],
                            tv, axis=mybir.AxisListType.X, op=add)
            if debug != "A":
                for t in range(NT):
                    yn = p2pool.tile([128, C * XS], f32, tag="yn")
                    nc.vector.tensor_tensor(yn[:, :], ybufs[t][:, :],
                                            nfx2ts[t][:, :], mult)
                    ostf = p2pool.tile([128, C], f32, tag="ostf")
                    nc.vector.tensor_reduce(
                        ostf[:, :],
                        yn[:, :].rearrange("p (c x) -> p c x", x=XS),
                        axis=mybir.AxisListType.X, op=add)
                    nc.sync.dma_start(out=outp[t * 128:(t + 1) * 128, :],
                                      in_=ostf[:, :])
    nc.finalize()
    return nc


def _prep_inputs(node_feats, w3, w2, w1, U3, U2, U1):
    """Host-side sharding / re-layout. No reference contractions are done
    here -- only transposes, padding, concatenation and dtype casts of the
    raw inputs."""
    import ml_dtypes
    bf16 = ml_dtypes.bfloat16
    f32 = np.float32
    node_feats = np.ascontiguousarray(np.asarray(node_feats, dtype=f32))
    w3 = np.asarray(w3, dtype=f32)
    w2 = np.asarray(w2, dtype=f32)
    w1 = np.asarray(w1, dtype=f32)
    U3 = np.asarray(U3, dtype=f32)
    U2 = np.asarray(U2, dtype=f32)
    U1 = np.asarray(U1, dtype=f32)

    # shared across cores.  w3p/w21 columns are permuted to c2-major order
    # (c' = (c%2)*48 + c//2) so the phase-A PSUM partitions line up with
    # the per-c-pair scratch blocks without splitting the partition dim.
    cperm = np.array([2 * (cc % PAIRS) + cc // PAIRS for cc in range(C)])
    w3p = np.zeros((KP, C), dtype=bf16)
    w3p[:K3] = w3
    w3p = np.ascontiguousarray(w3p[:, cperm])
    w21 = np.zeros((32, C), dtype=f32)
    w21[:K2] = w2
    w21[K2:K2 + K1] = w1
    w21 = np.ascontiguousarray(w21[:, cperm])

    # nfa: [p, cp, b]; p = 64*(c%2) + i'; i'=48 row is the ones channel
    nfT = node_feats.transpose(1, 2, 0)  # [c, i, b]
    nfa = np.zeros((128, PAIRS, B), dtype=f32)
    for par in (0, 1):
        nfa[64 * par:64 * par + I] = nfT[par::2].transpose(1, 0, 2)
        nfa[64 * par + I] = 1.0
    nfa = np.ascontiguousarray(nfa.reshape(128, PAIRS * B)).astype(bf16)

    U3h = U3.astype(bf16)  # cast once; per-core slices below stay bf16
    in_maps = []
    for r in range(NCORES):
        xlo = XS * r
        # u3t: [k, m], m = (i, x, y') with zero-padded y'=48 col and k pad
        u3s = U3h[xlo:xlo + XS]                      # [6, 48, 48, 1270]
        u3a = np.zeros((I, XS, Y1, KP), dtype=bf16)  # [i, x, y', k]
        u3a[:, :, :I, :K3] = u3s.transpose(2, 0, 1, 3)
        u3t = np.ascontiguousarray(u3a.reshape(MP, KP).T)

        # u2aug: rows 0:24 U2 slice, rows 24:27 U1 slice (at y'=48)
        u2a = np.zeros((32, XS, Y1), dtype=f32)
        u2a[:K2, :, :I] = U2[xlo:xlo + XS].transpose(2, 0, 1)
        u2a[K2:K2 + K1, :, I] = U1[xlo:xlo + XS].T
        u2a = np.ascontiguousarray(u2a.reshape(32, NX))

        # nfy: [b, c, 49] = nf with ones channel; nfx2: [b, c, 6] x-slice
        nfy = np.empty((B, C, I1), dtype=bf16)
        nfy[:, :, :I] = node_feats
        nfy[:, :, I] = 1.0
        nfy = np.ascontiguousarray(nfy.reshape(B, C * I1))
        nfx2 = np.ascontiguousarray(
            node_feats[:, :, xlo:xlo + XS].reshape(B, C * XS))

        in_maps.append({
            "u3t": u3t,
            "w3p": w3p,
            "nfa": nfa,
            "nfy": nfy,
            "nfx2": nfx2,
            "u2aug": u2a,
            "w21": w21,
        })
    return in_maps


def kernel(node_feats, w3, w2, w1, U3, U2, U1):
    global LAST_EXEC_NS
    import os
    from concourse.bass_utils import run_bass_kernel_spmd

    if "nc" not in _CACHE:
        _CACHE["nc"] = _build_nc()
    nc = _CACHE["nc"]

    in_maps = _prep_inputs(node_feats, w3, w2, w1, U3, U2, U1)
    trace = bool(os.environ.get("BASS_TRACE"))
    res = run_bass_kernel_spmd(nc, in_maps, list(range(NCORES)), trace=trace)
    LAST_EXEC_NS = res.exec_time_ns
    _CACHE["last_results"] = res

    out = np.zeros((B, C), dtype=np.float64)
    for r in range(NCORES):
        out += res.results[r]["out"].astype(np.float64)
    return out.astype(np.float32)



# revision 38
# speedup vs baseline: 1.0572x; 1.0513x over previous
"""Trainium2 Bass kernel for the MACE-style symmetric contraction:

    out  = einsum("xyik,kc,bci->bcxy", U3, w3, nf)
    c2   = einsum("xyk,kc->cxy", U2, w2)[None] + out
    out  = einsum("bcxi,bci->bcx", c2, nf)
    c1   = einsum("xk,kc->cx", U1, w1)[None] + out
    out  = einsum("bci,bci->bc", c1, nf)

Algebraically this is

    out[b,c] =   sum_{x,y,i} W3U[x,y,i,c] nf[b,c,x] nf[b,c,y] nf[b,c,i]
               + sum_{x,y}   U2w2[c,x,y]  nf[b,c,x] nf[b,c,y]
               + sum_{x}     U1w1[c,x]    nf[b,c,x]

with W3U = einsum("xyik,kc->xyic", U3, w3).  We fold the U2/U1 terms into
the triple product by augmenting the i axis (row i'=48 holds U2w2, entry
(i'=48, y'=48) holds U1w1) and appending a constant-1 channel to nf.

Sharding: the leading irrep axis x (48) is split 6-per-core across the 8
NeuronCores (this also splits the dominant HBM traffic, U3 = 562 MB, and
the W3U build FLOPs 8 ways).  Each core computes a partial [512, 96]
output (its x-slice of the outer sums); the host adds the 8 partials.

Per-core device pipeline:
  1. build W3Ua[c, m]  (m = (i,x,y') = 48*6*49) = u3t.T @ w3 on TensorE,
     PSUM -> DRAM scratch.
  2. re-layout per c-pair via DRAM->SBUF DMA into [i'=49(x2), (x,y')=294]
     tiles (partition rows 0:49 / 64:113).
  3. per (c, b-chunk of 128): one matmul  Z[b,(x,y')] = nfa_c.T @ W3Ua_c
     (contract i'), one VectorE multiply forming P2[b,(x,y')] =
     nf_x * nfa_y', and one fused tensor_tensor_reduce
     sum_{x,y'}(Z * P2) -> out[b, c].
"""

import numpy as np

B = 512          # atoms
C = 96           # feats
I = 48           # irreps
K3, K2, K1 = 1270, 24, 3
NCORES = 8
XS = I // NCORES  # 6 x-values per core
Y1 = I + 1        # 49: y plus augmentation column
I1 = I + 1        # 49: i plus augmentation row
KP = 1280         # K3 padded to 10 partition tiles
NX = XS * Y1      # 294
MP = I * XS * Y1  # 14112  (m = (i, x, y'), i outermost)
MCHUNK = 3 * NX   # 882: whole i-rows per chunk (16 chunks exactly)
NMC = MP // MCHUNK                 # 16
KT = KP // 128                     # 10
PAIRS = C // 2                     # 48
NF = 55                            # per-c nf pack: 49 (nfa_y) + 6 (nf_x)

_CACHE = {}

# exec time of the last device run (ns), when BASS_TRACE=1
LAST_EXEC_NS = None


def _build_nc(debug=None):
    import concourse.bass as bass
    import concourse.mybir as mybir
    from concourse.tile import TileContext

    f32 = mybir.dt.float32
    bf16 = mybir.dt.bfloat16
    mult = mybir.AluOpType.mult
    add = mybir.AluOpType.add

    import concourse.bacc as bacc
    nc = bacc.Bacc(None, target_bir_lowering=False)
    u3t = nc.dram_tensor("u3t", [KP, MP], bf16, kind="ExternalInput")
    w3p = nc.dram_tensor("w3p", [KP, C], bf16, kind="ExternalInput")
    nfa = nc.dram_tensor("nfa", [128, PAIRS * B], bf16, kind="ExternalInput")
    nfy = nc.dram_tensor("nfy", [B, C * I1], bf16, kind="ExternalInput")
    nfx2 = nc.dram_tensor("nfx2", [B, C * XS], f32, kind="ExternalInput")
    u2aug = nc.dram_tensor("u2aug", [32, NX], f32, kind="ExternalInput")
    w21 = nc.dram_tensor("w21", [32, C], f32, kind="ExternalInput")
    outp = nc.dram_tensor("out", [B, C], f32, kind="ExternalOutput")

    with TileContext(nc) as tc:
        with (
            tc.tile_pool(name="dram", bufs=1, space="DRAM") as dpool,
            tc.tile_pool(name="const", bufs=1) as cpool,
            tc.tile_pool(name="u3", bufs=10) as u3pool,
            tc.tile_pool(name="bpsum", bufs=4, space="PSUM") as bpsum,
            tc.tile_pool(name="zpsum", bufs=2, space="PSUM") as zpsum,
            tc.tile_pool(name="lt", bufs=3) as ltpool,
            tc.tile_pool(name="zsb", bufs=3) as zsbpool,
            tc.tile_pool(name="p2", bufs=3) as p2pool,
            tc.tile_pool(name="stg", bufs=3) as stgpool,
        ):
            # scratch laid out for phase-B consumption: per c-pair a
            # 128-row block (row = 64*(c%2) + i'), xy contiguous.
            w3u_scr = dpool.tile([PAIRS * 128, NX], bf16)
            scr_v = w3u_scr[:, :].rearrange("(cp c2 q) x -> cp c2 q x",
                                            c2=2, q=64)

            # ---- resident constants ----
            w3sb = cpool.tile([128, KT * C], bf16)
            w3v = w3sb[:, :].rearrange("p (k c) -> p k c", c=C)
            nc.sync.dma_start(
                out=w3v[:, :, :],
                in_=w3p[:, :].rearrange("(k p) c -> p k c", p=128))
            nfasb = cpool.tile([128, PAIRS * B], bf16)
            nc.sync.dma_start(out=nfasb[:, :], in_=nfa[:, :])
            nfav = nfasb[:, :].rearrange("p (cp b) -> p cp b", b=B)
            w21sb = cpool.tile([32, C], f32)
            nc.sync.dma_start(out=w21sb[:, :], in_=w21[:, :])
            u2sb = cpool.tile([32, NX], f32)
            nc.sync.dma_start(out=u2sb[:, :], in_=u2aug[:, :])
            NT = B // 128
            nfyts = [cpool.tile([128, C * I1], bf16, tag=f"nfy{t}",
                                name=f"nfy{t}") for t in range(NT)]
            for t in range(NT):
                nc.sync.dma_start(out=nfyts[t][:, :],
                                  in_=nfy[t * 128:(t + 1) * 128, :])
            ybufs = [cpool.tile([128, C * XS], f32, tag=f"yb{t}",
                                name=f"yb{t}") for t in range(NT)]
            nfx2ts = [cpool.tile([128, C * XS], f32, tag=f"nfx2{t}",
                                 name=f"nfx2{t}") for t in range(NT)]
            for t in range(NT):
                nc.sync.dma_start(out=nfx2ts[t][:, :],
                                  in_=nfx2[t * 128:(t + 1) * 128, :])

            # ---- aug build: [96, 294] = w21.T @ u2aug -> i'=48 rows ----
            aps = bpsum.tile([C, MCHUNK // 2], f32, tag="bp")
            nc.tensor.matmul(aps[:, :NX], w21sb[:27, :], u2sb[:27, :],
                             start=True, stop=True)
            astg = stgpool.tile([C, MCHUNK], bf16, tag="stg")
            nc.scalar.copy(astg[:, :NX], aps[:, :NX])
            # stg partitions are c2-major (w3p cols permuted host-side), so
            # each c2 half drains with a plain single-partition-dim DMA
            for c2 in range(2):
                nc.sync.dma_start(
                    out=scr_v[:, c2, I, :],
                    in_=astg[48 * c2:48 * (c2 + 1), :NX])

            # ---- W3U build: [96, 14112] = w3p.T @ u3t, k-accumulated ----
            # each chunk covers 3 whole i'-rows (882 = 3*294) so the drain
            # scatters straight into the per-c-pair scratch blocks
            MH = MCHUNK // 2  # 441: matmul free dim must stay <= 512
            for mc in range(NMC):
                subs = []
                for j in range(KT // 2):
                    tj = u3pool.tile([128, 2 * MCHUNK], bf16, tag="u3")
                    nc.sync.dma_start(
                        out=tj[:, :].rearrange("p (k m) -> p k m", k=2),
                        in_=u3t[256 * j:256 * (j + 1),
                                mc * MCHUNK:(mc + 1) * MCHUNK].rearrange(
                                    "(k p) m -> p k m", p=128))
                    subs.append(tj)
                stg = stgpool.tile([C, MCHUNK], bf16, tag="stg")
                for h in range(2):
                    ps = bpsum.tile([C, MH], f32, tag="bp",
                                    name=f"bp{mc}h{h}")
                    for kt in range(KT):
                        tv_ = subs[kt // 2][:, :].rearrange(
                            "p (k m) -> p k m", k=2)[
                            :, kt % 2, h * MH:(h + 1) * MH]
                        nc.tensor.matmul(ps[:, :], w3v[:, kt, :], tv_,
                                         start=(kt == 0),
                                         stop=(kt == KT - 1))
                    if mc % 2 == 0:
                        nc.scalar.copy(stg[:, h * MH:(h + 1) * MH],
                                       ps[:, :])
                    else:
                        nc.vector.tensor_copy(
                            stg[:, h * MH:(h + 1) * MH], ps[:, :])
                for c2 in range(2):
                    nc.sync.dma_start(
                        out=scr_v[:, c2, 3 * mc:3 * mc + 3, :],
                        in_=stg[48 * c2:48 * (c2 + 1), :].rearrange(
                            "cp (i x) -> cp i x", x=NX))

            npairs = 0 if debug == "A" else (
                debug if isinstance(debug, int) else PAIRS)
            # engine split: Scalar drains Z to bf16 SBUF (GpSimd cannot
            # read PSUM), the multiply runs on GpSimd for 3 of 4 c-pairs
            # and on DVE (2x bf16 mode) for the rest, DVE does every
            # reduce.
            # four channels (two c-pairs) per elementwise op: halves the
            # TT/RED instruction count and their fixed bubbles.  ~1/3 of
            # the multiplies run on DVE (2x bf16 mode), the rest on
            # GpSimd; DVE does every reduce.
            for cpp in range((npairs + 1) // 2):
                lt = ltpool.tile([128, 2 * NX], bf16, tag="lt")
                ltv = lt[:, :].rearrange("p (cpx x) -> p cpx x", x=NX)
                nc.sync.dma_start(
                    out=ltv,
                    in_=w3u_scr[256 * cpp:256 * (cpp + 1), :].rearrange(
                        "(cpx r) x -> r cpx x", cpx=2))
                c0 = 4 * cpp
                for t in range(NT):
                    zsb = zsbpool.tile([128, 4 * NX], bf16, tag="zsb")
                    for cpx in range(2):
                        cp = 2 * cpp + cpx
                        zt = zpsum.tile([128, 1024], f32, tag="z")
                        for ci in range(2):
                            lhsT = nfav[64 * ci:64 * ci + I1, cp,
                                        t * 128:(t + 1) * 128]
                            nc.tensor.matmul(
                                zt[:, 512 * ci:512 * ci + NX], lhsT,
                                ltv[64 * ci:64 * ci + I1, cpx, :],
                                start=True, stop=True)
                        nc.scalar.copy(
                            zsb[:, 2 * NX * cpx:2 * NX * (cpx + 1)],
                            zt[:, :].rearrange(
                                "p (c n) -> p c n", n=512)[:, :, 0:NX])
                    nfyv = nfyts[t][:, :].rearrange(
                        "p (c i) -> p c i", i=I1)[:, c0:c0 + 4, :]
                    zsv = zsb[:, :].rearrange("p (c x y) -> p c x y",
                                              c=4, y=Y1)
                    tmp = p2pool.tile([128, 4 * NX], bf16, tag="p2")
                    tv = tmp[:, :].rearrange("p (c x y) -> p c x y",
                                             c=4, y=Y1)
                    tt_eng = nc.vector if cpp % 3 == 2 else nc.gpsimd
                    tt_eng.tensor_tensor(
                        tv, zsv,
                        nfyv[:, :, None, :].to_broadcast(
                            [128, 4, XS, Y1]),
                        mult)
                    nc.vector.tensor_reduce(
                        ybufs[t][:, c0 * XS:(c0 + 4) * XS],
                        tv, axis=mybir.AxisListType.X, op=add)
            if debug != "A":
                for t in range(NT):
                    yn = p2pool.tile([128, C * XS], f32, tag="yn")
                    nc.vector.tensor_tensor(yn[:, :], ybufs[t][:, :],
                                            nfx2ts[t][:, :], mult)
                    ostf = p2pool.tile([128, C], f32, tag="ostf")
                    nc.vector.tensor_reduce(
                        ostf[:, :],
                        yn[:, :].rearrange("p (c x) -> p c x", x=XS),
                        axis=mybir.AxisListType.X, op=add)
                    nc.sync.dma_start(out=outp[t * 128:(t + 1) * 128, :],
                                      in_=ostf[:, :])
    nc.finalize()
    return nc


def _prep_inputs(node_feats, w3, w2, w1, U3, U2, U1):
    """Host-side sharding / re-layout. No reference contractions are done
    here -- only transposes, padding, concatenation and dtype casts of the
    raw inputs."""
    import ml_dtypes
    bf16 = ml_dtypes.bfloat16
    f32 = np.float32
    node_feats = np.ascontiguousarray(np.asarray(node_feats, dtype=f32))
    w3 = np.asarray(w3, dtype=f32)
    w2 = np.asarray(w2, dtype=f32)
    w1 = np.asarray(w1, dtype=f32)
    U3 = np.asarray(U3, dtype=f32)
    U2 = np.asarray(U2, dtype=f32)
    U1 = np.asarray(U1, dtype=f32)

    # shared across cores.  w3p/w21 columns are permuted to c2-major order
    # (c' = (c%2)*48 + c//2) so the phase-A PSUM partitions line up with
    # the per-c-pair scratch blocks without splitting the partition dim.
    cperm = np.array([2 * (cc % PAIRS) + cc // PAIRS for cc in range(C)])
    w3p = np.zeros((KP, C), dtype=bf16)
    w3p[:K3] = w3
    w3p = np.ascontiguousarray(w3p[:, cperm])
    w21 = np.zeros((32, C), dtype=f32)
    w21[:K2] = w2
    w21[K2:K2 + K1] = w1
    w21 = np.ascontiguousarray(w21[:, cperm])

    # nfa: [p, cp, b]; p = 64*(c%2) + i'; i'=48 row is the ones channel
    nfT = node_feats.transpose(1, 2, 0)  # [c, i, b]
    nfa = np.zeros((128, PAIRS, B), dtype=f32)
    for par in (0, 1):
        nfa[64 * par:64 * par + I] = nfT[par::2].transpose(1, 0, 2)
        nfa[64 * par + I] = 1.0
    nfa = np.ascontiguousarray(nfa.reshape(128, PAIRS * B)).astype(bf16)

    U3h = U3.astype(bf16)  # cast once; per-core slices below stay bf16
    in_maps = []
    for r in range(NCORES):
        xlo = XS * r
        # u3t: [k, m], m = (i, x, y') with zero-padded y'=48 col and k pad
        u3s = U3h[xlo:xlo + XS]                      # [6, 48, 48, 1270]
        u3a = np.zeros((I, XS, Y1, KP), dtype=bf16)  # [i, x, y', k]
        u3a[:, :, :I, :K3] = u3s.transpose(2, 0, 1, 3)
        u3t = np.ascontiguousarray(u3a.reshape(MP, KP).T)

        # u2aug: rows 0:24 U2 slice, rows 24:27 U1 slice (at y'=48)
        u2a = np.zeros((32, XS, Y1), dtype=f32)
        u2a[:K2, :, :I] = U2[xlo:xlo + XS].transpose(2, 0, 1)
        u2a[K2:K2 + K1, :, I] = U1[xlo:xlo + XS].T
        u2a = np.ascontiguousarray(u2a.reshape(32, NX))

        # nfy: [b, c, 49] = nf with ones channel; nfx2: [b, c, 6] x-slice
        nfy = np.empty((B, C, I1), dtype=bf16)
        nfy[:, :, :I] = node_feats
        nfy[:, :, I] = 1.0
        nfy = np.ascontiguousarray(nfy.reshape(B, C * I1))
        nfx2 = np.ascontiguousarray(
            node_feats[:, :, xlo:xlo + XS].reshape(B, C * XS))

        in_maps.append({
            "u3t": u3t,
            "w3p": w3p,
            "nfa": nfa,
            "nfy": nfy,
            "nfx2": nfx2,
            "u2aug": u2a,
            "w21": w21,
        })
    return in_maps


def kernel(node_feats, w3, w2, w1, U3, U2, U1):
    global LAST_EXEC_NS
    import os
    from concourse.bass_utils import run_bass_kernel_spmd

    if "nc" not in _CACHE:
        _CACHE["nc"] = _build_nc()
    nc = _CACHE["nc"]

    in_maps = _prep_inputs(node_feats, w3, w2, w1, U3, U2, U1)
    trace = bool(os.environ.get("BASS_TRACE"))
    res = run_bass_kernel_spmd(nc, in_maps, list(range(NCORES)), trace=trace)
    LAST_EXEC_NS = res.exec_time_ns
    _CACHE["last_results"] = res

    out = np.zeros((B, C), dtype=np.float64)
    for r in range(NCORES):
        out += res.results[r]["out"].astype(np.float64)
    return out.astype(np.float32)

